# revision 26
# baseline (speedup 1.0000x reference)
# Trainium2 Bass kernel for nn_Network_515396076038 (nms_detection / OICR-style loss).
#
# v2: the cross-core AllGather collective (45us round-trip on HW) is replaced
# by a direct peer-SBUF exchange built on remote_dma_broadcast:
#   - each core broadcasts a [128, 8] f32 payload (per-class max vm, winner
#     box+area candidates, z / s1 partial sums, class index on the PARTITION
#     axis) to all 8 cores with XOR-relative addressing -- SPMD-safe, and the
#     resulting per-core slot permutation is irrelevant because every
#     cross-core combine here (max, is-equal select, sum) is order-independent.
#   - arrival is gated by the broadcasts' remote semaphore (7 peers x 2 incs);
#     the wait is emitted as wait_ge(xr, 0) so the Tile scheduling sim doesn't
#     deadlock, then patched to 14 post-scheduling. A gpsimd self-copy of the
#     receive buffer after the wait makes every later consumer order correctly
#     through Tile's own dependency tracking.
#   - a fire-and-forget 1-element AllGather keeps NRT's collectives runtime
#     (and its CC-core entry barrier) alive, which also bounds cross-core
#     start skew long before the exchange triggers.
#   - the combine runs on the free axis (slots) with all 128 DVE lanes:
#     global vm = max over slots, winner = is_equal-masked sum, z/s1 = sum.
#     Winner boxes transpose back to a [1, 205] row and one broadcast matmul
#     replicates them to all partitions for the batched IoU phase.
# GEMM heads (fp8 DoubleRow, host-packed [128, KT, NS]) are unchanged from v1
# but reordered: cls|r1 head, then r2 (roi-resident, fills the PE while
# frm/ctx still stream from HBM), then the det head right behind its DMA.
# Scalar-engine activations are ordered Identity, Identity, Exp, Exp, Exp,
# Ln, Ln, Reciprocal so every ACT table switch hides off the critical path.
import sys

for _p in ("/opt/trn_rl_repo",):
    if _p not in sys.path:
        sys.path.append(_p)

import ml_dtypes
import numpy as np

import concourse.bass as bass
import concourse.bass_isa as bass_isa
from concourse.instruction_name_ordered_set import InstructionNameOrderedSet
import concourse.mybir as mybir
import concourse.tile as tile
from concourse import bacc
from concourse.bass_utils import run_bass_kernel_spmd
from concourse.masks import make_identity

dt = mybir.dt
Alu = mybir.AluOpType
Act = mybir.ActivationFunctionType
AX = mybir.AxisListType

C = 20       # foreground classes
CR = C + 1   # refine head classes (background + C)
CW = C + CR  # stacked critical-path roi heads: cls | r1 = 41
PA = 48      # w_a padded cols (fp8 DoubleRow needs 16B-aligned k-stride)
PD = 48      # w_det padded cols: W(20) | -W(20) | pad(8)
PR = 32      # w_r2 padded cols
PW = 6       # exchange payload cols: vm | x1 y1 x2 y2 area


def _emit(nc, tc, aps, NS, F, n_cores):
    NB = NS // 128   # 4 roi blocks
    KT = F // 128    # 32 contraction slices
    KH = KT // 2     # frm/ctx half chunk
    group = [list(range(n_cores))]

    (roi, frm, ctxm, w_a, w_det, w_r2, b_a, b_r2, bxw, lab, out) = aps

    const = tc.alloc_tile_pool(name="const", bufs=1)
    st = tc.alloc_tile_pool(name="st", bufs=1)
    pst = tc.alloc_tile_pool(name="pst", bufs=2, space="PSUM")
    pss = tc.alloc_tile_pool(name="pss", bufs=2, space="PSUM")
    psa = tc.alloc_tile_pool(name="psa", bufs=1, space="PSUM")
    dp = tc.alloc_tile_pool(name="dp", bufs=1, space="DRAM")
    psc = tc.alloc_tile_pool(name="psc", bufs=1, space="PSUM")

    # ---------------- cross-core exchange plumbing -------------------------
    xr = nc.alloc_semaphore("xr")        # remote arrival (7 peers x 2)
    xl = nc.alloc_semaphore("xl")        # local send-complete (unused)
    pay_sem = nc.alloc_semaphore("pay_sem")
    dum = nc.alloc_semaphore("dum")      # never incremented; gate placeholder

    pay = st.tile([128, PW], dt.float32)
    gsb = st.tile([128, n_cores, PW], dt.float32)

    # fire-and-forget tiny AllGather: the CC comm init it forces at NEFF
    # load/entry is REQUIRED for the remote DMA routes (dropping it faults
    # the device); nothing ever reads g1_out.
    g1_in = dp.tile([1], dt.float32)
    g1_out = dp.tile([n_cores], dt.float32)
    nc.gpsimd.collective_compute(
        "AllGather", Alu.bypass, replica_groups=group,
        ins=[g1_in.opt()], outs=[g1_out.opt()],
    )
    iota_i = const.tile([128, CR], dt.int32)
    nc.gpsimd.iota(iota_i, pattern=[[1, CR]], base=0, channel_multiplier=0)

    nc.vector.memset(pay, 0.0)
    # early descgen (reads pay AFTER the memset in Tile's view; the DMA only
    # reads SBUF at trigger time, so later payload writes are what ship).
    # u64 bitcast halves the D2D packet count (the fabric moves one packet
    # per element); sends spread over all 4 SWDGE queues to run in parallel.
    for k in range(1, n_cores):
        rd = [None] * 8
        rd[k] = (0, k)
        nc.gpsimd.remote_dma_broadcast(
            gsb[:, k, :].bitcast(dt.uint64), pay.bitcast(dt.uint64),
            xr, xl, rdests=rd, queue_num=(k - 1) % 4)

    # ---------------- DMA issue order = arrival order ----------------------
    fc7_sb = st.tile([128, 3 * KT, NS], dt.float8e4)   # roi | frm | ctx
    w_a_sb = const.tile([128, KT, PA], dt.float8e4)
    nc.sync.dma_start(w_a_sb, w_a)
    nc.sync.dma_start(fc7_sb[:, 0:KT, :], roi)
    w_r2_sb = const.tile([128, KT, PR], dt.float8e4)
    nc.sync.dma_start(w_r2_sb, w_r2)
    w_det_sb = const.tile([128, KT, PD], dt.float8e4)
    nc.sync.dma_start(w_det_sb, w_det)
    b_a_sb = const.tile([CW, 1], dt.float32)
    nc.sync.dma_start(b_a_sb, b_a[:, None])
    b_r2_sb = const.tile([CR, 1], dt.float32)
    nc.sync.dma_start(b_r2_sb, b_r2[:, None])
    bxw_sb = st.tile([128, NB, 5], dt.float32)
    nc.sync.dma_start(bxw_sb, bxw)
    labrow_i = st.tile([1, C], dt.int32)
    nc.sync.dma_start(labrow_i, lab)
    nc.sync.dma_start(fc7_sb[:, KT:KT + KH, :], frm[:, 0:KH, :])
    nc.sync.dma_start(fc7_sb[:, 2 * KT:2 * KT + KH, :], ctxm[:, 0:KH, :])
    nc.sync.dma_start(fc7_sb[:, KT + KH:2 * KT, :], frm[:, KH:KT, :])
    nc.sync.dma_start(fc7_sb[:, 2 * KT + KH:3 * KT, :], ctxm[:, KH:KT, :])
    boxes_nat = bxw_sb[:, :, 0:4]
    isw_col = bxw_sb[:, :, 4:5]     # [128, NB, 1]

    # ---------------- constants ----------------
    ident = const.tile([128, 128], dt.float32)
    make_identity(nc, ident)
    ones_col = const.tile([128, 1], dt.float32)
    nc.vector.memset(ones_col, 1.0)
    ones_row = const.tile([1, 128], dt.float32)
    nc.vector.memset(ones_row, 1.0)
    iota_f = const.tile([128, CR], dt.float32)
    nc.vector.tensor_copy(iota_f, iota_i)
    iota_m1k = const.tile([128, C], dt.float32)
    nc.vector.tensor_scalar_add(iota_m1k, iota_f[:, :C], -1000.0)
    warmid = const.tile([1, 1], dt.float32)
    nc.scalar.activation(warmid, ones_col[0:1, :], Act.Identity)  # warm table

    labrow_f = st.tile([1, C], dt.float32)
    nc.vector.tensor_copy(labrow_f, labrow_i)

    # roi areas + boxes|area pack for the sel gather (early, off critical path)
    ab_all = st.tile([128, NB, 1], dt.float32)
    tw = st.tile([128, NB, 1], dt.float32)
    nc.vector.tensor_tensor(ab_all, bxw_sb[:, :, 2:3], bxw_sb[:, :, 0:1], Alu.subtract)
    nc.vector.tensor_scalar_add(ab_all, ab_all, 1.0)
    nc.vector.tensor_tensor(tw, bxw_sb[:, :, 3:4], bxw_sb[:, :, 1:2], Alu.subtract)
    nc.vector.tensor_scalar_add(tw, tw, 1.0)
    nc.vector.tensor_mul(ab_all, ab_all, tw)
    bxa = st.tile([128, NB, 5], dt.float32)
    nc.vector.tensor_copy(bxa[:, :, 0:4], boxes_nat)
    nc.vector.tensor_copy(bxa[:, :, 4:5], ab_all)

    # per-class 1e30 mask for negative classes, on the class-partition axis:
    # [41, 1] with 1e30 at partitions {c, 21+c : label[c] != 1}; added to the
    # winner AREA so negative classes' IoU collapses to ~0.
    bm41row = st.tile([1, CW], dt.float32)
    nc.vector.memset(bm41row, 0.0)
    nc.vector.tensor_scalar(bm41row[:, 0:C], labrow_f, 1.0, 1e30, Alu.is_lt, Alu.mult)
    nc.vector.tensor_copy(bm41row[:, CR:CW], bm41row[:, 0:C])
    ps_bm = pst.tile([128, 128], dt.float32, tag="pt")
    nc.tensor.transpose(ps_bm[0:CW, 0:1], bm41row, ident[0:1, 0:1])
    bm41 = st.tile([CW, 1], dt.float32)
    nc.vector.tensor_copy(bm41, ps_bm[0:CW, 0:1])

    # ---------------- GEMM heads (fp8 DoubleRow) ---------------------------
    # PE order: cls|r1 (roi, streams behind its DMA), r2 (roi already
    # resident -- fills the PE while frm/ctx stream), det last (right behind
    # the frm/ctx DMA tail). The fast DoubleRow path (one 512-col pass for
    # both k-slices, ~216ns) only engages for <=32 weight cols: r2 padded to
    # 24 cols hits it; the 41-col cls|r1 head runs the 2-pass 427ns path.
    MA = CW   # w_a matmul cols
    MR = 24   # w_r2 matmul cols (junk out rows 21:24)
    scoresA = psc.tile([128, NS], dt.float32)   # rows 0:CW = cls | r1
    scoresB = psc.tile([128, NS], dt.float32)   # rows 0:C  = det (frm - ctx)
    DR = mybir.MatmulPerfMode.DoubleRow
    for j in range(KT // 2):
        nc.tensor.matmul(scoresA[0:MA, :], w_a_sb[:, 2 * j:2 * j + 2, 0:MA],
                         fc7_sb[:, 2 * j:2 * j + 2, :],
                         start=(j == 0), stop=(j == KT // 2 - 1), perf_mode=DR)
    scoresR = psa.tile([128, 512], dt.float32, tag="acc")
    for j in range(KT // 2):
        nc.tensor.matmul(scoresR[0:MR, :], w_r2_sb[:, 2 * j:2 * j + 2, 0:MR],
                         fc7_sb[:, 2 * j:2 * j + 2, :],
                         start=(j == 0), stop=(j == KT // 2 - 1), perf_mode=DR)

    # roi-head activations + transposes fill the PE before det's DMA lands
    CD = CW + C  # 61 packed roi-major columns
    # scalar queue: Identity x2 first, then all Exp, then Ln, then Reciprocal
    sA = st.tile([CW, NS], dt.float32)
    nc.scalar.activation(sA, scoresA[0:CW, :], Act.Identity, bias=b_a_sb)
    r2c = st.tile([CR, NS], dt.float32)
    nc.scalar.activation(r2c, scoresR[0:CR, :], Act.Identity, bias=b_r2_sb)

    rme = st.tile([128, NB, CD], dt.float32)     # cls|r1|exp(det), roi-major
    for b in range(NB):
        bsl = bass.ts(b, 128)
        pta = pst.tile([128, 128], dt.float32, tag="pt")
        nc.tensor.transpose(pta[:, 0:CW], sA[:, bsl], ident[0:CW, 0:CW])
        nc.vector.tensor_copy(rme[:, b, 0:CW], pta[:, 0:CW])
    rm = rme[:, :, 0:CW]
    ed = rme[:, :, CW:CD]

    for h in range(2):
        for j in range(KT // 4):
            kk = h * KH + 2 * j
            nc.tensor.matmul(scoresB[0:C, :], w_det_sb[:, kk:kk + 2, 0:C],
                             fc7_sb[:, KT + kk:KT + kk + 2, :],
                             start=(kk == 0), stop=False, perf_mode=DR)
        for j in range(KT // 4):
            kk = h * KH + 2 * j
            nc.tensor.matmul(scoresB[0:C, :], w_det_sb[:, kk:kk + 2, C:2 * C],
                             fc7_sb[:, 2 * KT + kk:2 * KT + kk + 2, :],
                             start=False, stop=(kk == KT - 2), perf_mode=DR)

    e = st.tile([128, NB, CW], dt.float32)       # exp(cls)|exp(r1)
    nc.scalar.activation(e, rm, Act.Exp)
    ec = e[:, :, 0:C]
    er = e[:, :, C:CW]
    scls = st.tile([128, NB, 1], dt.float32)
    nc.vector.reduce_sum(scls, ec, axis=AX.X)
    sr1 = st.tile([128, NB, 1], dt.float32)
    nc.vector.reduce_sum(sr1, er, axis=AX.X)
    rb1 = st.tile([128, NB, 1], dt.float32)
    nc.vector.reciprocal(rb1, scls)
    nc.vector.tensor_mul(rb1, rb1, isw_col)
    rb2 = st.tile([128, NB, 1], dt.float32)
    nc.vector.reciprocal(rb2, sr1)
    nc.vector.tensor_mul(rb2, rb2, isw_col)

    # det exp: scalar Exp directly follows e's Exp (no table switch)
    dE = st.tile([C, NS], dt.float32)
    nc.scalar.activation(dE, scoresB[0:C, :], Act.Exp)
    for b in range(NB):
        bsl = bass.ts(b, 128)
        ptd = pst.tile([128, 128], dt.float32, tag="pt")
        nc.tensor.transpose(ptd[:, 0:C], dE[:, bsl], ident[0:C, 0:C])
        nc.vector.tensor_copy(rme[:, b, CW:CD], ptd[:, 0:C])

    pq = st.tile([128, NB, CW], dt.float32)      # p1(20) | q2(21)
    nc.vector.tensor_mul(pq[:, :, 0:C], ec, ed)
    nc.vector.tensor_tensor(
        pq[:, :, 0:C], pq[:, :, 0:C], rb1.to_broadcast([128, NB, C]), Alu.mult)
    nc.vector.tensor_tensor(
        pq[:, :, C:CW], er, rb2.to_broadcast([128, NB, CR]), Alu.mult)

    # z / s1 partial sums: [1, 40] row via ones-matmul over rois
    zsp = st.tile([128, NB, 2 * C], dt.float32)  # exp(det) | cls*exp(det)
    nc.vector.tensor_copy(zsp[:, :, 0:C], ed)
    nc.vector.tensor_mul(zsp[:, :, C:2 * C], rm[:, :, 0:C], ed)
    ps_z = pss.tile([128, 512], dt.float32, tag="mm")
    for b in range(NB):
        nc.tensor.matmul(ps_z[0:1, 0:2 * C], ones_col, zsp[:, b, :],
                         start=(b == 0), stop=(b == NB - 1))
    zrow = st.tile([1, 2 * C], dt.float32)
    nc.vector.tensor_copy(zrow, ps_z[0:1, 0:2 * C])

    # per-class max over rois: free-dim max over blocks, then a GpSimd
    # cross-partition all-reduce (result is already broadcast to all rows)
    pmax = st.tile([128, CW], dt.float32)
    nc.vector.tensor_reduce(pmax, pq.rearrange("p b c -> p c b"),
                            axis=AX.X, op=Alu.max)
    vmP = st.tile([128, CW], dt.float32)
    nc.gpsimd.partition_all_reduce(
        vmP, pmax, channels=128, reduce_op=bass_isa.ReduceOp.max)

    sel = st.tile([128, NB, CW], dt.float32)
    nc.vector.tensor_tensor(
        sel, pq, vmP[:, None, :].to_broadcast([128, NB, CW]), Alu.is_equal)
    psq = psa.tile([128, 512], dt.float32, tag="acc")
    for b in range(NB):
        nc.tensor.matmul(psq[0:5, 0:CW], bxa[:, b, :], sel[:, b, :],
                         start=(b == 0), stop=(b == NB - 1))
    cand_sb = st.tile([5, CW], dt.float32)       # winner boxes+areas
    nc.vector.tensor_copy(cand_sb, psq[0:5, 0:CW])

    # ---------------- payload build + exchange trigger ---------------------
    # pp[0:41, 0] = vm, [0:41, 1:6] = box+area; z/s1 partials go to the host
    # directly (summed in finish()), so they stay out of the exchange.
    pp = pss.tile([128, 512], dt.float32, tag="mm")
    nc.tensor.transpose(pp[0:CW, 0:1], vmP[0:1, :], ident[0:1, 0:1])
    nc.tensor.transpose(pp[0:CW, 1:6], cand_sb, ident[0:5, 0:5])
    nc.vector.tensor_copy(pay[0:CW, 0:6], pp[0:CW, 0:6])
    nc.vector.tensor_copy(gsb[:, 0, :], pay)     # own slot, local copy

    # data-ready fence without a semaphore: a gpsimd READ of pay gets a real
    # Tile-managed cross-engine sync against the payload writes; the triggers
    # then follow same-engine via explicit scheduling-order edges (the Tile
    # scheduler reorders anything without an edge).
    payrd = st.tile([1, 1], dt.float32)
    paygate = nc.gpsimd.tensor_copy(payrd, pay[0:1, 0:1])
    trig_names = InstructionNameOrderedSet()
    for q in range(4):
        tq = nc.gpsimd.trigger_dma(count=None, queue_num=q)
        deps = InstructionNameOrderedSet()
        deps.add(paygate.ins.name)
        tq.ins.add_nosync_dependencies_from(deps)
        trig_names.add(tq.ins.name)

    # ---- exchange-latency filler: r2 transposes + both heads' log-softmax -
    r2m = st.tile([128, NB, CR], dt.float32)
    for b in range(NB):
        bsl = bass.ts(b, 128)
        ptr = pst.tile([128, 128], dt.float32, tag="pt")
        nc.tensor.transpose(ptr[:, 0:CR], r2c[:, bsl], ident[0:CR, 0:CR])
        nc.vector.tensor_copy(r2m[:, b, :], ptr[:, 0:CR])
    er2 = st.tile([128, NB, CR], dt.float32)
    nc.scalar.activation(er2, r2m, Act.Exp)
    sr2 = st.tile([128, NB, 1], dt.float32)
    nc.vector.reduce_sum(sr2, er2, axis=AX.X)

    # log-probs for both supervisions: x - ln(sum exp x); |scores| < ~4 so no
    # max-subtraction is needed
    xs = st.tile([128, NB, 2, CR], dt.float32)
    ln1 = st.tile([128, NB, 1], dt.float32)
    nc.scalar.activation(ln1, sr1, Act.Ln)
    nc.vector.tensor_tensor(
        xs[:, :, 0, :], rm[:, :, C:CW],
        ln1.to_broadcast([128, NB, CR]), Alu.subtract)
    ln2 = st.tile([128, NB, 1], dt.float32)
    nc.scalar.activation(ln2, sr2, Act.Ln)
    nc.vector.tensor_tensor(
        xs[:, :, 1, :], r2m, ln2.to_broadcast([128, NB, CR]), Alu.subtract)

    # ---------------- arrival gate + cross-core combine --------------------
    # The placeholder waits on a DUMMY sem (trivially satisfied, so the Tile
    # scheduling sim doesn't deadlock); build_program then adds xr >= 14 as
    # the EventSemaphore's second wait slot. Patching the same sem would be
    # silently dropped -- merge_waits dedups same-sem waits. The gpsimd
    # self-copy (ordered after the gate by an explicit edge) makes every
    # later gsb consumer order through Tile's own dependency tracking.
    gate = nc.gpsimd.wait_ge(dum, 0)
    # the gate must not hoist above the triggers (a blocked gate ahead of the
    # sends would deadlock all cores against each other)
    gate.ins.add_nosync_dependencies_from(trig_names)
    gflat = gsb.rearrange("p c w -> p (c w)")
    gcp = nc.gpsimd.tensor_copy(gflat, gflat)
    gdeps = InstructionNameOrderedSet()
    gdeps.add(gate.ins.name)
    gcp.ins.add_nosync_dependencies_from(gdeps)

    vmx = st.tile([128, 1], dt.float32)
    nc.vector.tensor_reduce(vmx, gsb[:, :, 0:1].rearrange("p s o -> p (o s)"),
                            axis=AX.X, op=Alu.max)
    sel8 = st.tile([128, n_cores], dt.float32)
    nc.vector.tensor_tensor(sel8, gsb[:, :, 0:1].rearrange("p s o -> p (o s)"),
                            vmx.to_broadcast([128, n_cores]), Alu.is_equal)
    wtmp = st.tile([128, n_cores, 5], dt.float32)
    nc.vector.tensor_tensor(
        wtmp, gsb[:, :, 1:6],
        sel8[:, :, None].to_broadcast([128, n_cores, 5]), Alu.mult)
    win = st.tile([128, 5], dt.float32)
    nc.vector.tensor_reduce(win, wtmp.rearrange("p s w -> p w s"),
                            axis=AX.X, op=Alu.add)
    nc.vector.tensor_tensor(win[0:CW, 4:5], win[0:CW, 4:5], bm41, Alu.add)

    # winner table -> one [1, 205] row -> broadcast to all partitions
    qrow = pss.tile([128, 512], dt.float32, tag="mm")
    for i in range(5):
        nc.tensor.transpose(qrow[0:1, CW * i:CW * i + CW],
                            win[0:CW, i:i + 1], ident[0:CW, 0:CW])
    qsb = st.tile([1, 5 * CW], dt.float32)
    nc.vector.tensor_copy(qsb, qrow[0:1, 0:5 * CW])
    QA = pss.tile([128, 512], dt.float32, tag="mm")
    nc.tensor.matmul(QA[:, 0:5 * CW], ones_row, qsb, start=True, stop=True)

    # ---------------- batched IoU / assignment / loss ----------------------
    W2 = CW  # 41-wide blocks; col 20 (bg) is junk and never read
    def qb(i):    # query coord block [128, 1, 41] -> [128, NB, 41]
        return QA[:, None, CW * i:CW * i + CW].to_broadcast([128, NB, W2])
    def bb(i):    # per-block roi coord [128, NB, 1] -> [128, NB, 41]
        return boxes_nat[:, :, i:i + 1].to_broadcast([128, NB, W2])

    xi1 = st.tile([128, NB, W2], dt.float32)
    nc.vector.tensor_tensor(xi1, qb(0), bb(0), Alu.max)
    yi1 = st.tile([128, NB, W2], dt.float32)
    nc.vector.tensor_tensor(yi1, qb(1), bb(1), Alu.max)
    xi2 = st.tile([128, NB, W2], dt.float32)
    nc.vector.tensor_tensor(xi2, qb(2), bb(2), Alu.min)
    yi2 = st.tile([128, NB, W2], dt.float32)
    nc.vector.tensor_tensor(yi2, qb(3), bb(3), Alu.min)
    nc.vector.tensor_tensor(xi2, xi2, xi1, Alu.subtract)
    nc.vector.tensor_scalar(xi2, xi2, 1.0, 0.0, Alu.add, Alu.max)   # iw
    nc.vector.tensor_tensor(yi2, yi2, yi1, Alu.subtract)
    nc.vector.tensor_scalar(yi2, yi2, 1.0, 0.0, Alu.add, Alu.max)   # ih
    inter = st.tile([128, NB, W2], dt.float32)
    nc.vector.tensor_mul(inter, xi2, yi2)
    un = st.tile([128, NB, W2], dt.float32)
    nc.vector.tensor_tensor(un, inter, ab_all.to_broadcast([128, NB, W2]),
                            Alu.subtract)
    nc.vector.tensor_tensor(un, qb(4), un, Alu.subtract)
    nc.vector.reciprocal(un, un)
    ov = st.tile([128, NB, W2], dt.float32)
    nc.vector.tensor_mul(ov, inter, un)

    # assignment + weighted log-prob, per supervision s (cols 0:20 / 21:41)
    stats = st.tile([128, 16], dt.float32)       # wl[8] | keep[8]
    wv = stats[:, 0:2 * NB].rearrange("p (b s) -> p b s", s=2)
    kv = stats[:, 2 * NB:4 * NB].rearrange("p (b s) -> p b s", s=2)
    for s, lo in ((0, 0), (1, CR)):
        ovs = ov[:, :, lo:lo + C]
        mo = st.tile([128, NB, 1], dt.float32)
        nc.vector.reduce_max(mo, ovs, axis=AX.X)
        meq = st.tile([128, NB, C], dt.float32)
        nc.vector.tensor_tensor(
            meq, ovs, mo.to_broadcast([128, NB, C]), Alu.is_equal)
        nc.vector.tensor_tensor(
            meq, meq, iota_m1k[:, None, :].to_broadcast([128, NB, C]), Alu.mult)
        gt = st.tile([128, NB, 1], dt.float32)
        nc.vector.tensor_reduce(gt, meq, axis=AX.X, op=Alu.min)
        nc.vector.tensor_scalar_add(gt, gt, 1001.0)          # argmax + 1
        fg = st.tile([128, NB, 1], dt.float32)
        nc.vector.tensor_scalar(fg, mo, 0.5, None, Alu.is_gt)
        keep = st.tile([128, NB, 1], dt.float32)
        nc.vector.tensor_scalar(keep, mo, 0.1, None, Alu.is_ge)
        col = st.tile([128, NB, 1], dt.float32)
        nc.vector.tensor_mul(col, gt, fg)                    # fg ? argmax+1 : 0
        oh = st.tile([128, NB, CR], dt.float32)
        nc.vector.tensor_tensor(
            oh, iota_f[:, None, :].to_broadcast([128, NB, CR]),
            col.to_broadcast([128, NB, CR]), Alu.is_equal)
        nc.vector.tensor_mul(oh, oh, xs[:, :, s, :])
        lpsel = st.tile([128, NB, 1], dt.float32)
        nc.vector.reduce_sum(lpsel, oh, axis=AX.X)
        wl = st.tile([128, NB, 1], dt.float32)
        nc.vector.tensor_mul(wl, keep, isw_col)
        nc.vector.tensor_mul(wl, wl, lpsel)
        nc.vector.tensor_copy(wv[:, :, s:s + 1], wl)
        nc.vector.tensor_copy(kv[:, :, s:s + 1], keep)

    ps_l = psa.tile([128, 512], dt.float32, tag="acc")
    nc.tensor.matmul(ps_l[0:16, 0:1], stats, ones_col, start=True, stop=True)
    lsum = st.tile([16, 1], dt.float32)
    nc.vector.tensor_copy(lsum, ps_l[0:16, 0:1])

    # hinge is finished on host from the per-core z|s1 partial sums
    nc.sync.dma_start(out[0:16], lsum[:, 0])
    nc.sync.dma_start(out[16:56], zrow)

    for pool in (psc, dp, psa, pss, pst, st, const):
        pool.release()
    return gate, xr


def build_program(NS=512, F=4096, n_cores=8):
    nc = bacc.Bacc(
        "TRN2", target_bir_lowering=False, debug=False, num_devices=n_cores,
        num_swdge_queues=4,
    )
    KT = F // 128
    roi = nc.dram_tensor("roi", [128, KT, NS], dt.float8e4, kind="ExternalInput").ap()
    frm = nc.dram_tensor("frm", [128, KT, NS], dt.float8e4, kind="ExternalInput").ap()
    ctxm = nc.dram_tensor("ctxm", [128, KT, NS], dt.float8e4, kind="ExternalInput").ap()
    w_a = nc.dram_tensor("w_a", [128, KT, PA], dt.float8e4, kind="ExternalInput").ap()
    w_det = nc.dram_tensor("w_det", [128, KT, PD], dt.float8e4, kind="ExternalInput").ap()
    w_r2 = nc.dram_tensor("w_r2", [128, KT, PR], dt.float8e4, kind="ExternalInput").ap()
    b_a = nc.dram_tensor("b_a", [CW], dt.float32, kind="ExternalInput").ap()
    b_r2 = nc.dram_tensor("b_r2", [CR], dt.float32, kind="ExternalInput").ap()
    bxw = nc.dram_tensor("bxw", [128, NS // 128, 5], dt.float32, kind="ExternalInput").ap()
    lab = nc.dram_tensor("lab", [1, C], dt.int32, kind="ExternalInput").ap()
    out = nc.dram_tensor("out", [56], dt.float32, kind="ExternalOutput").ap()
    aps = (roi, frm, ctxm, w_a, w_det, w_r2, b_a, b_r2, bxw, lab, out)
    with tile.TileContext(nc) as tc:
        gate, xr = _emit(nc, tc, aps, NS, F, n_cores)
    # patch the arrival gate to its real threshold (see _emit)
    gate.wait_op(xr, 2 * (n_cores - 1), "sem-ge", check=False)
    nc.compile()
    return nc


def _pack_fc7(a_t_bf16, sl, F):
    # [F, NS] slice -> [128, KT, NS] with contiguous per-partition runs
    return np.ascontiguousarray(
        a_t_bf16[:, sl].reshape(F // 128, 128, -1).transpose(1, 0, 2))


def _pack_w(w, pad_cols):
    F, cols = w.shape
    wp = np.zeros((F, pad_cols), np.float32)
    wp[:, 0:cols] = w
    return np.ascontiguousarray(
        wp.astype(ml_dtypes.float8_e4m3fn).reshape(F // 128, 128, pad_cols)
        .transpose(1, 0, 2))


def make_in_maps(inputs, NS, n_cores):
    f32 = np.float32
    fp8 = ml_dtypes.float8_e4m3fn
    w_a = _pack_w(np.concatenate(
        [np.asarray(inputs["W_cls"], f32), np.asarray(inputs["W_r1"], f32)], axis=1), PA)
    wd = np.asarray(inputs["W_det"], f32)
    w_det = _pack_w(np.concatenate([wd, -wd], axis=1), PD)
    w_r2 = _pack_w(np.asarray(inputs["W_r2"], f32), PR)
    b_a = np.ascontiguousarray(np.concatenate(
        [np.asarray(inputs["b_cls"]), np.asarray(inputs["b_r1"])]), f32)
    b_r2 = np.ascontiguousarray(np.asarray(inputs["b_r2"]), f32)
    boxes = np.asarray(inputs["ss_boxes"], f32)[:, 1:5]
    iswf = np.asarray(inputs["IS_weight"], f32)[:, 0]
    lab = np.ascontiguousarray(np.asarray(inputs["image_level_label"]), np.int32)
    roi = np.asarray(inputs["fc7_roi"], f32).T.astype(fp8)
    frm = np.asarray(inputs["fc7_frame"], f32).T.astype(fp8)
    ctxm = np.asarray(inputs["fc7_context"], f32).T.astype(fp8)
    F = roi.shape[0]
    NB = NS // 128

    in_maps = []
    for c in range(n_cores):
        sl = slice(c * NS, (c + 1) * NS)
        bsh = boxes[sl].reshape(NB, 128, 4).transpose(1, 0, 2)
        ish = iswf[sl].reshape(NB, 128).T[:, :, None]
        bxw = np.ascontiguousarray(np.concatenate([bsh, ish], axis=2), f32)
        in_maps.append({
            "roi": _pack_fc7(roi, sl, F),
            "frm": _pack_fc7(frm, sl, F),
            "ctxm": _pack_fc7(ctxm, sl, F),
            "w_a": w_a, "w_det": w_det, "w_r2": w_r2,
            "b_a": b_a, "b_r2": b_r2,
            "bxw": bxw, "lab": lab,
        })
    return in_maps


_PROG_CACHE = {}


def _get_prog(NS, F, n_cores):
    key = (NS, F, n_cores)
    if key not in _PROG_CACHE:
        _PROG_CACHE[key] = build_program(NS, F, n_cores)
    return _PROG_CACHE[key]


def build_repair(n_cores=8):
    # Device semaphores survive NEFF loads, and the main kernel can only
    # RESTORE its sems (decrement), not clear them. This prelude program
    # resets DMA state and clears the whole kernel sem range so the main
    # kernel starts from the zero state its waits assume.
    nc = bacc.Bacc("TRN2", target_bir_lowering=False, debug=False,
                   num_devices=n_cores)
    x = nc.dram_tensor("x", [1, 1], dt.float32, kind="ExternalInput").ap()
    out = nc.dram_tensor("out", [1, 1], dt.float32, kind="ExternalOutput").ap()
    with tile.TileContext(nc) as tc:
        st = tc.alloc_tile_pool(name="st", bufs=1)
        nc.gpsimd.dma_reset()
        nc.gpsimd.sem_clear(nc._kernel_sem_range)
        t = st.tile([1, 1], dt.float32)
        nc.sync.dma_start(t, x)
        nc.sync.dma_start(out, t)
        st.release()
    nc.compile()
    return nc


def run_repair(n_cores=8):
    key = ("repair", n_cores)
    if key not in _PROG_CACHE:
        _PROG_CACHE[key] = build_repair(n_cores)
    z = np.zeros((1, 1), np.float32)
    run_bass_kernel_spmd(_PROG_CACHE[key], [{"x": z}] * n_cores,
                         list(range(n_cores)))


def finish(results, lab, n_cores=8):
    # host-side gather/unshard: combine the per-core partial sums
    parts = np.stack([np.asarray(results[i]["out"], np.float64).reshape(56)
                      for i in range(n_cores)])
    wl = parts[:, 0:8].sum(axis=0)      # per (b, s=idx%2) weighted log-probs
    kp = parts[:, 8:16].sum(axis=0)     # per (b, s) keep counts
    rl1 = -wl[0::2].sum() / kp[0::2].sum()
    rl2 = -wl[1::2].sum() / kp[1::2].sum()
    z = parts[:, 16:36].sum(axis=0)
    s1 = parts[:, 36:56].sum(axis=0)
    h = np.maximum(0.0, 1.0 - lab * (s1 / z)).sum()
    return np.float32(h / C + 0.1 * rl1 + 0.1 * rl2)


def kernel(**inputs):
    n_cores = 8
    N, F = inputs["fc7_roi"].shape
    NS = N // n_cores
    prog = _get_prog(NS, F, n_cores)
    in_maps = make_in_maps(inputs, NS, n_cores)
    run_repair(n_cores)
    res = run_bass_kernel_spmd(prog, in_maps, list(range(n_cores))).results
    lab = np.asarray(inputs["image_level_label"], np.float64)[0]
    return finish(res, lab, n_cores)
